# revision 26
# baseline (speedup 1.0000x reference)
"""Cross-view attention (nn_CrossViewAttention) Trainium2 Bass kernel.

Reference computation (B=2, N=4096, D=512):
    co    = relu(concat([x_f, x_s], -1) @ Wc.T + bc)
    out_f = attend(x_f@Wq.T+bq, x_s@Wk.T+bk, x_f@Wv.T+bv) + co
    out_s = attend(x_s@Wq.T+bq, x_f@Wk.T+bk, x_s@Wv.T+bv) + co
    attend(Q,K,V) = (softmax(Q K^T) / L1 / sqrt(D)) @ V

Sharding: 8 cores = (direction f/s) x (batch 0/1) x (sequence half).
Each core computes 2048 output rows of one direction against the full
4096-row K/V for its (direction, batch), SPMD with per-core input data.
Rows are permuted host-side so every core's own rows come first; the
attention reduction over keys is permutation invariant.  K, V and co
are computed fully locally -- collectives proved to serialize the
whole kernel (~67us each in the cost model) for only ~41us of PE
savings, so there are none.

Design notes:
  - x arrives pre-transposed (feature-major) in bf16 from the host: no
    on-device PE transposes.
  - Q/K are projected in bf16 but stored fp8-e4m3; the N x N score
    matmul runs in fp8 DoubleRow mode (K=256 per instruction).
  - exp runs on ACT into bf16 tiles (scores reach ~71, far outside any
    fp8 range); row sums come from a ones-stationary matmul over those
    tiles; the reciprocals are broadcast across partitions via a DRAM
    bounce and multiplied in on DVE, giving L1-normalized
    probabilities in [0,1] that quantize safely to fp8-e5m2.  The PV
    matmul then runs in fp8 DoubleRow with e5m2 V, and needs no
    post-normalization: the output is one fused DVE op
    out = pv * (1/sqrt(D)) + co.
"""

import sys
from contextlib import ExitStack

for _p in ("/opt/trn_rl_repo", "/root/.axon_site/_ro/trn_rl_repo"):
    if _p not in sys.path:
        sys.path.insert(0, _p)

import ml_dtypes
import numpy as np

import concourse.bacc as bacc
import concourse.bass as bass
import concourse.mybir as mybir
import concourse.tile as tile

P = 128
D = 512
DC = D // P  # contraction chunks of 128
INV_SQRT_D = float(1.0 / np.sqrt(D))
EXP_SHIFT = -40.0

F32 = mybir.dt.float32
BF16 = mybir.dt.bfloat16
FP8_QK = mybir.dt.float8e4
FP8_AT = mybir.dt.float8e5
AF = mybir.ActivationFunctionType
DR = mybir.MatmulPerfMode.DoubleRow


def build_program(nq, nkv, reps=1, fp8_scores=True):
    nc = bacc.Bacc("TRN2", target_bir_lowering=False, debug=False, num_devices=8)

    # feature-major bf16 views, own query-half columns first
    xAT = nc.dram_tensor("xAT", [D, nkv], BF16, kind="ExternalInput").ap()
    xBT = nc.dram_tensor("xBT", [D, nkv], BF16, kind="ExternalInput").ap()
    wqT = nc.dram_tensor("wqT", [D, D], BF16, kind="ExternalInput").ap()
    wkT = nc.dram_tensor("wkT", [D, D], BF16, kind="ExternalInput").ap()
    wvT = nc.dram_tensor("wvT", [D, D], BF16, kind="ExternalInput").ap()
    wcAT = nc.dram_tensor("wcAT", [D, D], BF16, kind="ExternalInput").ap()
    wcBT = nc.dram_tensor("wcBT", [D, D], BF16, kind="ExternalInput").ap()
    bq = nc.dram_tensor("bq", [D], F32, kind="ExternalInput").ap()
    bk = nc.dram_tensor("bk", [D], F32, kind="ExternalInput").ap()
    bv = nc.dram_tensor("bv", [D], F32, kind="ExternalInput").ap()
    bc = nc.dram_tensor("bc", [D], F32, kind="ExternalInput").ap()
    out = nc.dram_tensor("out", [nq, D], F32, kind="ExternalOutput").ap()

    with tile.TileContext(nc) as tc:
        for rep in range(reps):
            _emit_body(
                nc, tc, xAT, xBT, wqT, wkT, wvT, wcAT, wcBT,
                bq, bk, bv, bc, out, nq, nkv, fp8_scores, rep,
            )

    nc.compile()
    return nc


def _emit_body(
    nc, tc, xAT, xBT, wqT, wkT, wvT, wcAT, wcBT,
    bq, bk, bv, bc, out, nq, nkv, fp8_scores, rep,
):
    NBQ = nq // P   # query row blocks (16)
    MCK = nkv // P  # key row chunks (32)
    qk_dt = FP8_QK if fp8_scores else BF16

    rs_dram = nc.dram_tensor(f"rs_dram_{rep}", [512], F32).ap()

    with ExitStack() as st:
        persist = st.enter_context(tc.tile_pool(name="persist", bufs=1))

        # K-side weights first: the first K projection matmuls gate phase 1
        w_sb = {}
        for nm, ap_ in (
            ("wk", wkT), ("wq", wqT), ("wv", wvT), ("wcA", wcAT), ("wcB", wcBT),
        ):
            t = persist.tile([P, DC, D], BF16, name=f"w_{nm}")
            nc.sync.dma_start(out=t, in_=ap_.rearrange("(c p) o -> p c o", p=P))
            w_sb[nm] = t

        bq_sb = persist.tile([P, DC], F32, name="bq_sb")
        bk_sb = persist.tile([P, DC], F32, name="bk_sb")
        for ob in range(DC):
            nc.sync.dma_start(
                out=bq_sb[:, ob : ob + 1], in_=bq[ob * P : (ob + 1) * P][:, None]
            )
            nc.sync.dma_start(
                out=bk_sb[:, ob : ob + 1], in_=bk[ob * P : (ob + 1) * P][:, None]
            )

        bv_bc = persist.tile([P, D], F32, name="bv_bc")
        nc.sync.dma_start(
            out=bv_bc,
            in_=bass.AP(tensor=bv.tensor, offset=bv.offset, ap=[[0, P]] + list(bv.ap)),
        )
        bc_bc = persist.tile([P, D], F32, name="bc_bc")
        nc.sync.dma_start(
            out=bc_bc,
            in_=bass.AP(tensor=bc.tensor, offset=bc.offset, ap=[[0, P]] + list(bc.ap)),
        )
        ones_col = persist.tile([P, 1], BF16, name="ones_col")
        nc.vector.memset(ones_col, 1.0)
        shift_sb = persist.tile([P, 1], F32, name="shift_sb")
        nc.vector.memset(shift_sb, EXP_SHIFT)

        qT_sb = persist.tile([P, DC, nq], qk_dt, name="qT_sb")
        kT_sb = persist.tile([P, DC, nkv], qk_dt, name="kT_sb")
        v_sb = persist.tile([P, MCK, D], FP8_AT, name="v_sb")
        co_sb = persist.tile([P, NBQ, D], BF16, name="co_sb")

        # ---------------- phase 1: projections (all local) ----------------
        with ExitStack() as ph1:
            xst_pool = ph1.enter_context(tc.tile_pool(name="xst", bufs=1))
            co_pool = ph1.enter_context(tc.tile_pool(name="cop", bufs=3))
            ps1 = ph1.enter_context(tc.tile_pool(name="ps1", bufs=4, space="PSUM"))

            # chunk the 4MB x loads along columns so the first K/Q matmuls
            # start as soon as the first column block lands (region-level
            # deps); B view first -- the K loop consumes it first
            xAT_sb = xst_pool.tile([P, DC, nkv], BF16, name="xAT_sb")
            xBT_sb = xst_pool.tile([P, DC, nkv], BF16, name="xBT_sb")
            for src, dst in ((xBT, xBT_sb), (xAT, xAT_sb)):
                src_r = src.rearrange("(c p) n -> p c n", p=P)
                for n0 in range(0, nkv, 1024):
                    nc.sync.dma_start(
                        out=dst[:, :, n0 : n0 + 1024],
                        in_=src_r[:, :, n0 : n0 + 1024],
                    )

            # K over all keys, then Q (scores need both; emitted first so
            # phase 2 can start as soon as V/co still stream behind them)
            for s0 in range(0, nkv, 512):
                for ob in range(DC):
                    ps = ps1.tile([P, 512], F32, name="ps_k", tag="ps1")
                    for c in range(DC):
                        nc.tensor.matmul(
                            ps,
                            lhsT=w_sb["wk"][:, c, ob * P : (ob + 1) * P],
                            rhs=xBT_sb[:, c, s0 : s0 + 512],
                            start=(c == 0),
                            stop=(c == DC - 1),
                        )
                    nc.scalar.activation(
                        out=kT_sb[:, ob, s0 : s0 + 512],
                        in_=ps,
                        func=AF.Identity,
                        bias=bk_sb[:, ob : ob + 1],
                        scale=1.0,
                    )
            for s0 in range(0, nq, 512):
                for ob in range(DC):
                    ps = ps1.tile([P, 512], F32, name="ps_q", tag="ps1")
                    for c in range(DC):
                        nc.tensor.matmul(
                            ps,
                            lhsT=w_sb["wq"][:, c, ob * P : (ob + 1) * P],
                            rhs=xAT_sb[:, c, s0 : s0 + 512],
                            start=(c == 0),
                            stop=(c == DC - 1),
                        )
                    nc.scalar.activation(
                        out=qT_sb[:, ob, s0 : s0 + 512],
                        in_=ps,
                        func=AF.Identity,
                        bias=bq_sb[:, ob : ob + 1],
                        scale=1.0,
                    )

            # V over all key rows of the A view, stored e5m2 for fp8 PV;
            # bv is NOT added here -- attention rows sum to 1 after
            # normalization, so bv enters as +bv/sqrt(D) via co instead.
            for m in range(MCK):
                ps = ps1.tile([P, 512], F32, name="ps_v", tag="ps1")
                for c in range(DC):
                    nc.tensor.matmul(
                        ps,
                        lhsT=xAT_sb[:, c, m * P : (m + 1) * P],
                        rhs=w_sb["wv"][:, c, :],
                        start=(c == 0),
                        stop=(c == DC - 1),
                    )
                nc.scalar.activation(out=v_sb[:, m, :], in_=ps, func=AF.Copy)

            # co = relu(xA@WcA.T + xB@WcB.T + bc) + bv/sqrt(D), own rows
            for nb in range(NBQ):
                ps = ps1.tile([P, 512], F32, name="ps_c", tag="ps1")
                for c in range(DC):
                    nc.tensor.matmul(
                        ps,
                        lhsT=xAT_sb[:, c, nb * P : (nb + 1) * P],
                        rhs=w_sb["wcA"][:, c, :],
                        start=(c == 0),
                        stop=False,
                    )
                for c in range(DC):
                    nc.tensor.matmul(
                        ps,
                        lhsT=xBT_sb[:, c, nb * P : (nb + 1) * P],
                        rhs=w_sb["wcB"][:, c, :],
                        start=False,
                        stop=(c == DC - 1),
                    )
                cadd = co_pool.tile([P, D], F32, name="cadd", tag="cadd")
                nc.vector.tensor_add(cadd, ps, bc_bc)
                crl = co_pool.tile([P, D], F32, name="crl", tag="crl")
                nc.scalar.activation(out=crl, in_=cadd, func=AF.Relu)
                nc.vector.tensor_add(co_sb[:, nb, :], crl, bv_bc)

        # ---------------- phase 2: attention (S^T layout) ----------------
        at_pool = st.enter_context(tc.tile_pool(name="at_pool", bufs=2))
        a8_pool = st.enter_context(tc.tile_pool(name="a8_pool", bufs=2))
        r_pool = st.enter_context(tc.tile_pool(name="r_pool", bufs=2))
        o_pool = st.enter_context(tc.tile_pool(name="o_pool", bufs=3))
        sps_pool = st.enter_context(tc.tile_pool(name="sps", bufs=3, space="PSUM"))
        sum_pool = st.enter_context(tc.tile_pool(name="sump", bufs=2, space="PSUM"))
        pv_pool = st.enter_context(tc.tile_pool(name="pv", bufs=2, space="PSUM"))

        for s0 in range(0, nq, 512):
            at_sb = at_pool.tile([P, MCK, 512], BF16, name="at_sb", tag="at")
            at8 = a8_pool.tile([P, MCK, 512], FP8_AT, name="at8", tag="at8")
            ssum = sum_pool.tile([1, 512], F32, name="ssum", tag="ssum")
            for mb in range(MCK):
                sps = sps_pool.tile([P, 512], F32, name="sps", tag="sps")
                if fp8_scores:
                    for c2 in range(DC // 2):
                        nc.tensor.matmul(
                            sps,
                            lhsT=kT_sb[:, 2 * c2 : 2 * c2 + 2, mb * P : (mb + 1) * P],
                            rhs=qT_sb[:, 2 * c2 : 2 * c2 + 2, s0 : s0 + 512],
                            start=(c2 == 0),
                            stop=(c2 == DC // 2 - 1),
                            perf_mode=DR,
                        )
                else:
                    for c in range(DC):
                        nc.tensor.matmul(
                            sps,
                            lhsT=kT_sb[:, c, mb * P : (mb + 1) * P],
                            rhs=qT_sb[:, c, s0 : s0 + 512],
                            start=(c == 0),
                            stop=(c == DC - 1),
                        )
                nc.scalar.activation(
                    out=at_sb[:, mb, :],
                    in_=sps,
                    func=AF.Exp,
                    bias=shift_sb,
                    scale=1.0,
                )
                # L1 row sums over keys, accumulated across key chunks
                nc.tensor.matmul(
                    ssum,
                    lhsT=ones_col,
                    rhs=at_sb[:, mb, :],
                    start=(mb == 0),
                    stop=(mb == MCK - 1),
                )
            # 1/rowsum, broadcast to all partitions via DRAM
            rs_row = r_pool.tile([1, 512], F32, name="rs_row", tag="rsr")
            nc.vector.reciprocal(out=rs_row, in_=ssum)
            nc.sync.dma_start(out=rs_dram, in_=rs_row)
            rs_bc = r_pool.tile([P, 512], F32, name="rs_bc", tag="rsb")
            nc.sync.dma_start(
                out=rs_bc,
                in_=bass.AP(
                    tensor=rs_dram.tensor,
                    offset=rs_dram.offset,
                    ap=[[0, P]] + list(rs_dram.ap),
                ),
            )
            # normalize -> fp8 probabilities
            for mb in range(MCK):
                nc.vector.tensor_mul(at8[:, mb, :], at_sb[:, mb, :], rs_bc)

            for j in range(4):
                pv = pv_pool.tile([P, D], F32, name="pv", tag="pv")
                for i2 in range(MCK // 2):
                    nc.tensor.matmul(
                        pv,
                        lhsT=at8[:, 2 * i2 : 2 * i2 + 2, j * P : (j + 1) * P],
                        rhs=v_sb[:, 2 * i2 : 2 * i2 + 2, :],
                        start=(i2 == 0),
                        stop=(i2 == MCK // 2 - 1),
                        perf_mode=DR,
                    )
                nb = s0 // P + j
                outt = o_pool.tile([P, D], F32, name="outt", tag="outt")
                nc.vector.scalar_tensor_tensor(
                    out=outt,
                    in0=pv,
                    scalar=INV_SQRT_D,
                    in1=co_sb[:, nb, :],
                    op0=mybir.AluOpType.mult,
                    op1=mybir.AluOpType.add,
                )
                nc.sync.dma_start(out=out[nb * P : (nb + 1) * P, :], in_=outt)


_PROG_CACHE = {}


def _get_program(nq, nkv):
    key = (nq, nkv)
    if key not in _PROG_CACHE:
        _PROG_CACHE[key] = build_program(nq, nkv)
    return _PROG_CACHE[key]


def make_in_maps(x_f, x_s, Wq, bq, Wk, bk, Wv, bv, Wc, bc):
    """Per-core SPMD input dicts + (direction, batch, half) layout.

    x is shipped pre-transposed (feature-major) in bf16, full 4096 rows
    per view with the core's own query-half rows first.
    """
    x_f = np.asarray(x_f, np.float32)
    x_s = np.asarray(x_s, np.float32)
    B, N, _ = x_f.shape
    nq = N // 2
    bf = ml_dtypes.bfloat16
    WqT = np.ascontiguousarray(np.asarray(Wq, np.float32).T).astype(bf)
    WkT = np.ascontiguousarray(np.asarray(Wk, np.float32).T).astype(bf)
    WvT = np.ascontiguousarray(np.asarray(Wv, np.float32).T).astype(bf)
    Wc = np.asarray(Wc, np.float32)
    WcfT = np.ascontiguousarray(Wc[:, :D].T).astype(bf)
    WcsT = np.ascontiguousarray(Wc[:, D:].T).astype(bf)
    bq32, bk32, bv32, bc32 = (
        np.ascontiguousarray(np.asarray(b, np.float32)) for b in (bq, bk, bv, bc)
    )
    # Attention rows sum to 1 after L1 normalization and the kernel scales by
    # 1/sqrt(D), so bv enters the output as bv/sqrt(D), added via co.
    bv32 = np.ascontiguousarray(bv32 / np.sqrt(D, dtype=np.float32))
    in_maps, layout = [], []
    for d in range(2):
        for b in range(B):
            for h in range(2):
                xq = x_f[b] if d == 0 else x_s[b]
                xk = x_s[b] if d == 0 else x_f[b]
                if h == 1:
                    idx = np.r_[nq:N, 0:nq]
                    xq, xk = xq[idx], xk[idx]
                in_maps.append(
                    {
                        "xAT": np.ascontiguousarray(xq.T).astype(bf),
                        "xBT": np.ascontiguousarray(xk.T).astype(bf),
                        "wqT": WqT,
                        "wkT": WkT,
                        "wvT": WvT,
                        "wcAT": WcfT if d == 0 else WcsT,
                        "wcBT": WcsT if d == 0 else WcfT,
                        "bq": bq32,
                        "bk": bk32,
                        "bv": bv32,
                        "bc": bc32,
                    }
                )
                layout.append((d, b, h))
    return in_maps, layout


def kernel(x_f, x_s, Wq, bq, Wk, bk, Wv, bv, Wc, bc):
    x_f = np.asarray(x_f, np.float32)
    B, N, _ = x_f.shape
    nq = N // 2
    nc = _get_program(nq, N)
    in_maps, layout = make_in_maps(x_f, x_s, Wq, bq, Wk, bk, Wv, bv, Wc, bc)

    from concourse.bass_utils import run_bass_kernel_spmd

    res = run_bass_kernel_spmd(nc, in_maps, list(range(len(in_maps))))
    out_f = np.empty((B, N, D), np.float32)
    out_s = np.empty((B, N, D), np.float32)
    for (d, b, h), r in zip(layout, res.results):
        tgt = out_f if d == 0 else out_s
        tgt[b, h * nq : (h + 1) * nq] = r["out"]
    return out_f, out_s
